# revision 1
# baseline (speedup 1.0000x reference)
"""LoRA attention kernel for Trainium2, batch-sharded across 8 NeuronCores.

Strategy:
  - Data parallel: batch B=8 -> one batch element per core.
  - LoRA factors are folded into Wqkv on the host (exact algebra, float64):
        q' = q @ (I + Aq Bq)  ==>  Wq' = (I + Aq Bq).T @ Wq   (per head)
  - All matmuls run as float32r (fp32 with 11-bit mantissa, full PE rate).
  - q,k are produced transposed ([head_dim, tokens]) directly from x^T so the
    score matmuls need no on-chip transposes. v is produced in natural layout
    with an extra all-ones column per head, so the attention-value matmul
    accumulates the softmax denominators for free in its last output row.
  - Scores are computed transposed, s[k, q]; softmax normalization is applied
    to the (small) attention output instead of the score matrix.
  - The output projection consumes the transposed attention output directly
    and produces y in natural layout; bias is fused into the PSUM drain.
  - Every matmul moving operand is a standalone tile with pitch == N
    (strided moving operands run at half rate on the PE).
"""
import numpy as np

import concourse.bass as bass
import concourse.bacc as bacc
import concourse.mybir as mybir
import concourse.tile as tile
from concourse.bass_utils import run_bass_kernel_spmd

F32 = mybir.dt.float32
F32R = mybir.dt.float32r
EXP = mybir.ActivationFunctionType.Exp

B, N, C, H, HD = 8, 1024, 768, 12, 64
CT = C // 128           # 6 contraction tiles over C
TT = N // 128           # 8 token tiles
QC = N // 512           # 2 query chunks of 512
KT = N // 128           # 8 key tiles of 128
EC = 2                  # output-projection feature chunks of 384
SCALE = HD ** -0.5
N_CORES = 8

_NC_CACHE = None


def _build():
    nc = bacc.Bacc(None, target_bir_lowering=False)

    xT = nc.dram_tensor("xT", [C, N], F32R, kind="ExternalInput")
    wqk = nc.dram_tensor("wqk", [H, CT, 128, 128], F32R, kind="ExternalInput")
    wv = nc.dram_tensor("wv", [CT, 128, C], F32R, kind="ExternalInput")
    wpt = nc.dram_tensor("wpt", [CT, 128, C], F32R, kind="ExternalInput")
    bias = nc.dram_tensor("bias", [1, C], F32, kind="ExternalInput")
    y = nc.dram_tensor("y", [N, C], F32, kind="ExternalOutput")

    from contextlib import ExitStack
    with tile.TileContext(nc) as tc:
        with ExitStack() as ctx:
            pool = lambda name, bufs, **kw: ctx.enter_context(
                tc.tile_pool(name=name, bufs=bufs, **kw))
            xt_pool = pool("xt", 2 * CT)
            wqk_pool = pool("wqkp", 2 * CT)
            w384_pool = pool("w384", 2 * CT)      # wv then wpt (disjoint phases)
            vaug_pool = pool("vaug", TT)
            st_pool = pool("stp", 6)
            kt_pool = pool("ktp", 6)
            exp_pool = pool("expp", 8)
            avs_pool = pool("avsp", 4)
            iv_pool = pool("ivp", 3)
            bc_pool = pool("bcp", 3)
            ost_pool = pool("ostp", 3)
            out_pool = pool("outp", CT)
            y_pool = pool("yp", 3)
            cst_pool = pool("cst", 1)
            proj_ps = pool("proj_ps", 3, space="PSUM")
            sc_ps = pool("sc_ps", 3, space="PSUM")
            av_ps = pool("av_ps", 2, space="PSUM")

            # ---- PE warm-up: dummy matmuls bridge the DMA lead-in so the
            # HAM clock gate opens before real work arrives -----------------
            wuf = cst_pool.tile([128, 512], F32, tag="wuf")
            nc.vector.memset(wuf, 0.0)
            wur = cst_pool.tile([128, 512], F32R, tag="wur")
            nc.vector.tensor_copy(wur, wuf)

            def warmup(n, label):
                for i in range(n):
                    wps = proj_ps.tile([128, 512], F32, tag="mmps",
                                       name=f"wu{label}_{i}")
                    nc.tensor.matmul(wps, wur[:, 0:128], wur,
                                     start=True, stop=True)

            warmup(30, "a")

            # ---- loads ---------------------------------------------------
            def load_wqk(h):
                wts = []
                for c in range(CT):
                    wt = wqk_pool.tile([128, 128], F32R, tag="wqk",
                                       name=f"wqk{h}_{c}")
                    nc.sync.dma_start(out=wt, in_=wqk[h, c, :, :])
                    wts.append(wt)
                return wts

            wts0 = load_wqk(0)

            # x^T in 12 standalone [128, 512] tiles (contiguous pitch)
            xt = [[None] * QC for _ in range(CT)]
            for c in range(CT):
                for qc in range(QC):
                    t = xt_pool.tile([128, 512], F32R, tag="xt",
                                     name=f"xt{c}_{qc}")
                    nc.sync.dma_start(
                        out=t, in_=xT[c * 128:(c + 1) * 128,
                                      qc * 512:(qc + 1) * 512])
                    xt[c][qc] = t

            bias_bc = cst_pool.tile([128, C], F32, tag="biasbc")
            nc.sync.dma_start(out=bias_bc, in_=bias[:, :].to_broadcast([128, C]))
            ones12 = cst_pool.tile([128, H], F32, tag="ones12")
            nc.vector.memset(ones12, 1.0)

            wvt = [[None] * 2 for _ in range(CT)]
            for c in range(CT):
                for half in range(2):
                    t = w384_pool.tile([128, 384], F32R, tag="w384",
                                       name=f"wv{c}_{half}")
                    nc.sync.dma_start(
                        out=t, in_=wv[c, :, half * 384:(half + 1) * 384])
                    wvt[c][half] = t

            # ---- per-head q/k projection ---------------------------------
            def qk_project(h, wts):
                """q (rows 0-63) and k (rows 64-127), transposed layout.
                Returns ([stA, stB], [ktA, ktB]) per 512-token chunk."""
                sts, kts = [], []
                for qc in range(QC):
                    st = st_pool.tile([128, 512], F32R, tag="st",
                                      name=f"st{h}_{qc}")
                    pqk = proj_ps.tile([128, 512], F32, tag="mmps",
                                       name=f"pqk{h}_{qc}")
                    for c in range(CT):
                        nc.tensor.matmul(
                            pqk, wts[c], xt[c][qc],
                            start=(c == 0), stop=(c == CT - 1),
                        )
                    nc.vector.tensor_copy(st, pqk)
                    # move k rows to the top of a base-0 tile (partition shift
                    # via DMA) and zero rows 64-127 so the score matmuls can
                    # run with K=128 (uniform PE tile config; zeros are exact)
                    kt_t = kt_pool.tile([128, 512], F32R, tag="kt",
                                        name=f"kt{h}_{qc}")
                    nc.sync.dma_start(out=kt_t[0:64, :], in_=st[64:128, :])
                    nc.vector.tensor_copy(kt_t[64:128, :], wur[64:128, :])
                    sts.append(st)
                    kts.append(kt_t)
                return sts, kts

            head0 = qk_project(0, wts0)

            # ---- v_aug[tt] = [v | 1] per head, natural layout ------------
            vaug = []
            for tt in range(TT):
                va = vaug_pool.tile([128, (H - 1) * 65 + 128], F32R,
                                    tag="vaug", name=f"vaug{tt}")
                for half in range(2):
                    pv = proj_ps.tile([128, 384], F32, tag="mmps",
                                      name=f"pv{tt}_{half}")
                    for c in range(CT):
                        nc.tensor.matmul(
                            pv,
                            xt[c][tt // 4][:, (tt % 4) * 128:(tt % 4 + 1) * 128],
                            wvt[c][half],
                            start=(c == 0), stop=(c == CT - 1),
                        )
                    dst = bass.AP(tensor=va.tensor,
                                  offset=va.offset + half * 6 * 65,
                                  ap=[va.ap[0], [65, 6], [1, 64]])
                    nc.vector.tensor_copy(dst, pv)
                ones_ap = bass.AP(tensor=va.tensor, offset=va.offset + 64,
                                  ap=[va.ap[0], [65, H]])
                nc.vector.tensor_copy(ones_ap, ones12)
                # zero the tail cols so the widened av lhsT reads no garbage
                nc.vector.tensor_copy(va[:, H * 65:], wur[:, 0:(H - 1) * 65 + 128 - H * 65])
                vaug.append(va)

            # ---- output accumulator tiles (c-major, [128, N]) ------------
            outT = [out_pool.tile([128, N], F32R, tag="outT", name=f"outT{i}")
                    for i in range(CT)]

            # ---- per-head attention --------------------------------------
            wptt = None
            last_avs = [None]
            head_order = list(range(H))
            head_order[10], head_order[11] = head_order[11], head_order[10]
            for h in head_order:
                sts, kts = head0 if h == 0 else qk_project(h, load_wqk(h))

                for qc in range(QC):
                    av = av_ps.tile([128, 512], F32, tag="av",
                                    name=f"av{h}_{qc}")
                    for kt in range(KT):
                        ps_s = sc_ps.tile([128, 512], F32, tag="sc",
                                          name=f"sc{h}_{qc}_{kt}")
                        nc.tensor.matmul(
                            ps_s,
                            kts[kt // 4][:, (kt % 4) * 128:(kt % 4 + 1) * 128],
                            sts[qc], start=True, stop=True,
                        )
                        et = exp_pool.tile([128, 512], F32R, tag="exp",
                                           name=f"exp{h}_{qc}_{kt}")
                        nc.scalar.activation(out=et, in_=ps_s, func=EXP,
                                             scale=SCALE)
                        nc.tensor.matmul(
                            av, vaug[kt][:, h * 65:h * 65 + 128], et,
                            start=(kt == 0), stop=(kt == KT - 1),
                        )
                    # drain the whole psum quickly to release the bank; the
                    # normalization then runs off the PE critical path
                    avs = avs_pool.tile([65, 512], F32, tag="avs",
                                        name=f"avs{h}_{qc}")
                    nc.vector.tensor_copy(avs, av[0:65, :])
                    last_avs[0] = avs
                    # row 64 of avs = softmax denominators for this q chunk.
                    # DMA-shift them to partition 0, then fast-reciprocal and
                    # broadcast (both require base partition 0).
                    sm0 = iv_pool.tile([1, 512], F32, tag="sm0",
                                       name=f"sm0{h}_{qc}")
                    nc.sync.dma_start(out=sm0, in_=avs[64:65, :])
                    iv0 = iv_pool.tile([1, 512], F32, tag="iv0",
                                       name=f"iv0{h}_{qc}")
                    nc.vector.reciprocal_approx_fast(out=iv0, in_=sm0)
                    bc = bc_pool.tile([64, 512], F32, tag="bc",
                                      name=f"bc{h}_{qc}")
                    nc.gpsimd.partition_broadcast(bc, iv0)

                    ct_i = h // 2
                    if h % 2 == 0:
                        nc.vector.tensor_mul(
                            outT[ct_i][0:64, qc * 512:(qc + 1) * 512],
                            avs[0:64, :], bc)
                    else:
                        ost = ost_pool.tile([64, 512], F32R, tag="ost",
                                            name=f"ost{h}_{qc}")
                        nc.vector.tensor_mul(ost, avs[0:64, :], bc)
                        nc.sync.dma_start(
                            out=outT[ct_i][64:128, qc * 512:(qc + 1) * 512],
                            in_=ost)

                if h == 5:
                    # prefetch output-projection weights mid-flight
                    wptt = [[None] * EC for _ in range(CT)]
                    for c in range(CT):
                        for ec in range(EC):
                            t = w384_pool.tile([128, 384], F32R, tag="w384",
                                               name=f"wpt{c}_{ec}")
                            nc.sync.dma_start(
                                out=t,
                                in_=wpt[c, :, ec * 384:(ec + 1) * 384])
                            wptt[c][ec] = t

            # ---- output projection ---------------------------------------
            for i in range(5):
                wps = proj_ps.tile([128, 512], F32, tag="mmps",
                                   name=f"wub_{i}")
                nc.tensor.matmul(wps, wuf[0:65, 0:128], last_avs[0],
                                 start=True, stop=True)
            for tt in range(TT):
                ysb = y_pool.tile([128, C], F32, tag="y", name=f"y{tt}")
                for ec in range(EC):
                    py = proj_ps.tile([128, 384], F32, tag="mmps",
                                      name=f"py{tt}_{ec}")
                    for c in range(CT):
                        nc.tensor.matmul(
                            py,
                            outT[c][:, tt * 128:(tt + 1) * 128],
                            wptt[c][ec],
                            start=(c == 0), stop=(c == CT - 1),
                        )
                    nc.vector.tensor_add(ysb[:, ec * 384:(ec + 1) * 384], py,
                                         bias_bc[:, ec * 384:(ec + 1) * 384])
                nc.sync.dma_start(out=y[tt * 128:(tt + 1) * 128, :], in_=ysb)

    nc.finalize()
    return nc


def _get_nc():
    global _NC_CACHE
    if _NC_CACHE is None:
        _NC_CACHE = _build()
    return _NC_CACHE


def _host_prep(x, Wqkv, Wproj, bproj, Aq, Bq, Av, Bv):
    """Fold LoRA into the weights and lay everything out for the kernel."""
    W = Wqkv.astype(np.float64)
    Wq = W[0:C].reshape(H, HD, C)
    Wk = W[C:2 * C].reshape(H, HD, C)
    Wv_ = W[2 * C:3 * C].reshape(H, HD, C)
    ABq = Aq.astype(np.float64) @ Bq.astype(np.float64)   # [HD, HD]
    ABv = Av.astype(np.float64) @ Bv.astype(np.float64)
    Wq = Wq + np.einsum('ed,hec->hdc', ABq, Wq)           # (I+AB).T @ Wq per head
    Wv_ = Wv_ + np.einsum('ed,hec->hdc', ABv, Wv_)

    # wqk[h, c] = [K=c-rows(128), M = q_h cols(64) ++ k_h cols(64)]
    wqk = np.empty((H, CT, 128, 128), np.float32)
    for h in range(H):
        for c in range(CT):
            cs = slice(c * 128, (c + 1) * 128)
            wqk[h, c, :, 0:64] = Wq[h][:, cs].T.astype(np.float32)
            wqk[h, c, :, 64:128] = Wk[h][:, cs].astype(np.float32).T

    # wv[c] = [K=c-rows(128), all 768 v output features]
    WvT = Wv_.reshape(C, C).T.astype(np.float32)          # [c_in, v_out]
    wv = np.ascontiguousarray(WvT.reshape(CT, 128, C))

    # wpt[c] = Wproj.T c-tiles: [K=c(128), e(768)]
    WpT = Wproj.astype(np.float32).T                      # [c, e]
    wpt = np.ascontiguousarray(WpT.reshape(CT, 128, C))

    bias = bproj.astype(np.float32).reshape(1, C)

    per_core = []
    for b in range(B):
        xTb = np.ascontiguousarray(x[b].astype(np.float32).T)   # [C, N]
        per_core.append({"xT": xTb, "wqk": wqk, "wv": wv, "wpt": wpt,
                         "bias": bias})
    return per_core


def kernel(x, Wqkv, Wproj, bproj, Aq, Bq, Av, Bv, _trace=False):
    x = np.asarray(x)
    in_maps = _host_prep(np.asarray(x), np.asarray(Wqkv), np.asarray(Wproj),
                         np.asarray(bproj), np.asarray(Aq), np.asarray(Bq),
                         np.asarray(Av), np.asarray(Bv))
    nc = _get_nc()
    res = run_bass_kernel_spmd(nc, in_maps, core_ids=list(range(N_CORES)),
                               trace=_trace)
    out = np.stack([res.results[b]["y"] for b in range(B)], axis=0)
    if _trace:
        kernel._last_result = res
    return out.astype(np.float32)



# revision 2
# speedup vs baseline: 1.0987x; 1.0987x over previous
"""LoRA attention kernel for Trainium2, batch-sharded across 8 NeuronCores.

Strategy (v2):
  - Data parallel: batch B=8 -> one batch element per core.
  - LoRA factors folded into Wqkv on the host (exact algebra, float64);
    the 1/sqrt(hd) score scale is folded into Wq as well.
  - All matmul operands are bfloat16 (1 cycle/row on the PE regardless of
    moving free dim, and FWL halves LDWEIGHTS time); accumulation is fp32
    in PSUM so only operand rounding is lost. rel err ~5e-3 << 2e-2 gate.
  - Heads are processed in pairs (2p, 2p+1). qT/kT tiles pack the pair's
    head dims on partitions [0:64] / [64:128]. The score matmuls are ROW
    TILED: two concurrent 64-row matmuls (tile_position auto-derived from
    base partitions) compute both heads' scores in one 512-column pass,
    doubling score throughput vs zero-padding K to 128.
  - The pair's two score outputs go to one [128,1024] 2-bank PSUM tile so
    a single ACT instruction applies exp to both (halves ACT bubbles).
  - v is produced in natural layout with an extra all-ones column per head
    (65-col pitch); the attention-value matmul (M=65) accumulates softmax
    denominators for free in its last output row.
  - Normalization runs off the PE critical path: DVE drains av PSUM,
    DMA shifts the denominator row to partition 0, fast reciprocal,
    gpsimd partition-broadcast, DVE multiply into outT.
  - Output projection consumes outT (c-major, bf16) directly; bias fused
    into the PSUM drain. Projection for the first query half is emitted
    right after the last pair so it fills PE gaps during the final
    ACT-paced stretch.
"""
import numpy as np
import ml_dtypes

import concourse.bass as bass
import concourse.bacc as bacc
import concourse.mybir as mybir
import concourse.tile as tile
from concourse.bass_utils import run_bass_kernel_spmd

F32 = mybir.dt.float32
BF16 = mybir.dt.bfloat16
EXP = mybir.ActivationFunctionType.Exp

B, N, C, H, HD = 8, 1024, 768, 12, 64
P = H // 2              # 6 head pairs
CT = C // 128           # 6 contraction tiles over C
QC = N // 512           # 2 query chunks of 512
KT = N // 128           # 8 key tiles of 128
EC = 2                  # output-projection feature chunks of 384
VP = HD + 1             # 65-col pitch per head in vaug
N_CORES = 8

_NC_CACHE = None


def _build():
    nc = bacc.Bacc(None, target_bir_lowering=False)

    xT = nc.dram_tensor("xT", [C, N], BF16, kind="ExternalInput")
    # wqk[p, 0/1(q/k), c] = [K=c-rows(128), M=(head 2p dims 64 | head 2p+1 dims 64)]
    wqk = nc.dram_tensor("wqk", [P, 2, CT, 128, 128], BF16, kind="ExternalInput")
    wv = nc.dram_tensor("wv", [CT, 128, C], BF16, kind="ExternalInput")
    wpt = nc.dram_tensor("wpt", [CT, 128, C], BF16, kind="ExternalInput")
    bias = nc.dram_tensor("bias", [1, C], F32, kind="ExternalInput")
    y = nc.dram_tensor("y", [N, C], F32, kind="ExternalOutput")

    from contextlib import ExitStack
    with tile.TileContext(nc) as tc:
        with ExitStack() as ctx:
            pool = lambda name, bufs, **kw: ctx.enter_context(
                tc.tile_pool(name=name, bufs=bufs, **kw))
            xt_pool = pool("xt", 2 * CT)
            wqk_pool = pool("wqkp", 4 * CT)       # 2 pairs in flight x (q+k)
            w384_pool = pool("w384", 2 * CT)      # wv then wpt (disjoint phases)
            vaug_pool = pool("vaug", KT)
            qt_pool = pool("qtp", 4)              # qT pair tiles, 2 pairs
            kt_pool = pool("ktp", 4)
            et_pool = pool("expp", 4)
            avs_pool = pool("avsp", 4)
            iv_pool = pool("ivp", 4)
            bc_pool = pool("bcp", 3)
            ost_pool = pool("ostp", 3)
            out_pool = pool("outp", 2 * CT)
            y_pool = pool("yp", 3)
            cst_pool = pool("cst", 1)
            proj_ps = pool("proj_ps", 2, space="PSUM")
            sc_ps = pool("sc_ps", 2, space="PSUM")
            av_ps = pool("av_ps", 2, space="PSUM")

            # ---- PE warm-up: dummy matmuls bridge the DMA lead-in so the
            # HAM clock gate opens before real work arrives -----------------
            wuf = cst_pool.tile([128, 512], F32, tag="wuf")
            nc.vector.memset(wuf, 0.0)
            wur = cst_pool.tile([128, 512], BF16, tag="wur")
            nc.vector.tensor_copy(wur, wuf)

            def warmup(n, label):
                for i in range(n):
                    wps = proj_ps.tile([128, 512], F32, tag="mmps",
                                       name=f"wu{label}_{i}")
                    nc.tensor.matmul(wps, wur[:, 0:128], wur,
                                     start=True, stop=True)

            warmup(14, "a")

            # ---- loads ---------------------------------------------------
            def load_wqk(p):
                wts = []
                for qk in range(2):
                    for c in range(CT):
                        wt = wqk_pool.tile([128, 128], BF16, tag="wqk",
                                           name=f"wqk{p}_{qk}_{c}")
                        nc.sync.dma_start(out=wt, in_=wqk[p, qk, c, :, :])
                        wts.append(wt)
                return wts  # [q c0..c5, k c0..c5]

            wts0 = load_wqk(0)

            # x^T in 12 standalone [128, 512] bf16 tiles
            xt = [[None] * QC for _ in range(CT)]
            for c in range(CT):
                for qc in range(QC):
                    t = xt_pool.tile([128, 512], BF16, tag="xt",
                                     name=f"xt{c}_{qc}")
                    nc.sync.dma_start(
                        out=t, in_=xT[c * 128:(c + 1) * 128,
                                      qc * 512:(qc + 1) * 512])
                    xt[c][qc] = t

            bias_bc = cst_pool.tile([128, C], F32, tag="biasbc")
            nc.sync.dma_start(out=bias_bc, in_=bias[:, :].to_broadcast([128, C]))
            ones12 = cst_pool.tile([128, H], BF16, tag="ones12")
            nc.vector.memset(ones12, 1.0)

            wvt = [[None] * 2 for _ in range(CT)]
            for c in range(CT):
                for half in range(2):
                    t = w384_pool.tile([128, 384], BF16, tag="w384",
                                       name=f"wv{c}_{half}")
                    nc.sync.dma_start(
                        out=t, in_=wv[c, :, half * 384:(half + 1) * 384])
                    wvt[c][half] = t

            # ---- per-pair q/k projection ---------------------------------
            def qk_project(p, wts):
                """qT_pair / kT_pair: [128 = (h dims | h' dims), 512 tokens]
                per 512-token chunk, bf16."""
                qts, kts = [], []
                for qk in range(2):
                    for qc in range(QC):
                        dst_pool = qt_pool if qk == 0 else kt_pool
                        st = dst_pool.tile([128, 512], BF16, tag="st",
                                           name=f"st{p}_{qk}_{qc}")
                        pqk = proj_ps.tile([128, 512], F32, tag="mmps",
                                           name=f"pqk{p}_{qk}_{qc}")
                        for c in range(CT):
                            nc.tensor.matmul(
                                pqk, wts[qk * CT + c], xt[c][qc],
                                start=(c == 0), stop=(c == CT - 1),
                            )
                        nc.vector.tensor_copy(st, pqk)
                        (qts if qk == 0 else kts).append(st)
                return qts, kts

            pair0 = qk_project(0, wts0)

            # ---- v_aug[kt] = [v | 1] per head, natural layout ------------
            vaug = []
            for tt in range(KT):
                va = vaug_pool.tile([128, H * VP], BF16,
                                    tag="vaug", name=f"vaug{tt}")
                for half in range(2):
                    pv = proj_ps.tile([128, 384], F32, tag="mmps",
                                      name=f"pv{tt}_{half}")
                    for c in range(CT):
                        nc.tensor.matmul(
                            pv,
                            xt[c][tt // 4][:, (tt % 4) * 128:(tt % 4 + 1) * 128],
                            wvt[c][half],
                            start=(c == 0), stop=(c == CT - 1),
                        )
                    dst = bass.AP(tensor=va.tensor,
                                  offset=va.offset + half * 6 * VP,
                                  ap=[va.ap[0], [VP, 6], [1, HD]])
                    nc.vector.tensor_copy(dst, pv)
                ones_ap = bass.AP(tensor=va.tensor, offset=va.offset + HD,
                                  ap=[va.ap[0], [VP, H]])
                nc.vector.tensor_copy(ones_ap, ones12)
                vaug.append(va)

            # ---- output accumulator tiles (c-major, [128, 512] per qc) ---
            outT = [[out_pool.tile([128, 512], BF16, tag="outT",
                                   name=f"outT{i}_{qc}")
                     for qc in range(QC)] for i in range(CT)]

            # ---- per-pair attention --------------------------------------
            wptt = None
            for p in range(P):
                qts, kts = pair0 if p == 0 else qk_project(p, load_wqk(p))
                h0, h1 = 2 * p, 2 * p + 1

                for qc in range(QC):
                    av0 = av_ps.tile([VP, 512], F32, tag="av",
                                     name=f"av{h0}_{qc}")
                    av1 = av_ps.tile([VP, 512], F32, tag="av",
                                     name=f"av{h1}_{qc}")
                    for kt in range(KT):
                        # row-tiled pair scores -> one 2-bank psum tile
                        ps_s = sc_ps.tile([128, 1024], F32, tag="sc",
                                          name=f"sc{p}_{qc}_{kt}")
                        klhs = kts[kt // 4][:, (kt % 4) * 128:(kt % 4 + 1) * 128]
                        nc.tensor.matmul(
                            ps_s[:, 0:512], klhs[0:64, :], qts[qc][0:64, :],
                            start=True, stop=True,
                        )
                        nc.tensor.matmul(
                            ps_s[:, 512:1024], klhs[64:128, :],
                            qts[qc][64:128, :],
                            start=True, stop=True,
                        )
                        et = et_pool.tile([128, 1024], BF16, tag="exp",
                                          name=f"exp{p}_{qc}_{kt}")
                        nc.scalar.activation(out=et, in_=ps_s, func=EXP)
                        nc.tensor.matmul(
                            av0, vaug[kt][:, h0 * VP:h0 * VP + VP],
                            et[:, 0:512],
                            start=(kt == 0), stop=(kt == KT - 1),
                        )
                        nc.tensor.matmul(
                            av1, vaug[kt][:, h1 * VP:h1 * VP + VP],
                            et[:, 512:1024],
                            start=(kt == 0), stop=(kt == KT - 1),
                        )

                    # drain + normalize, off the PE critical path
                    for hi, av in ((0, av0), (1, av1)):
                        h = 2 * p + hi
                        avs = avs_pool.tile([VP, 512], F32, tag="avs",
                                            name=f"avs{h}_{qc}")
                        nc.vector.tensor_copy(avs, av)
                        # row 64 = softmax denominators; shift to partition 0
                        sm0 = iv_pool.tile([1, 512], F32, tag="sm0",
                                           name=f"sm0{h}_{qc}")
                        nc.sync.dma_start(out=sm0, in_=avs[HD:VP, :])
                        iv0 = iv_pool.tile([1, 512], F32, tag="iv0",
                                           name=f"iv0{h}_{qc}")
                        nc.vector.reciprocal_approx_fast(out=iv0, in_=sm0)
                        bc = bc_pool.tile([64, 512], F32, tag="bc",
                                          name=f"bc{h}_{qc}")
                        nc.gpsimd.partition_broadcast(bc, iv0)
                        if hi == 0:
                            nc.vector.tensor_mul(
                                outT[p][qc][0:64, :], avs[0:HD, :], bc)
                        else:
                            ost = ost_pool.tile([64, 512], BF16, tag="ost",
                                                name=f"ost{h}_{qc}")
                            nc.vector.tensor_mul(ost, avs[0:HD, :], bc)
                            nc.sync.dma_start(out=outT[p][qc][64:128, :],
                                              in_=ost)

                if p == 2:
                    # prefetch output-projection weights mid-flight
                    wptt = [[None] * EC for _ in range(CT)]
                    for c in range(CT):
                        for ec in range(EC):
                            t = w384_pool.tile([128, 384], BF16, tag="w384",
                                               name=f"wpt{c}_{ec}")
                            nc.sync.dma_start(
                                out=t,
                                in_=wpt[c, :, ec * 384:(ec + 1) * 384])
                            wptt[c][ec] = t

            # ---- output projection (qc0 tokens first: fills PE gaps while
            # the last pair's qc1 attention is still ACT-paced) ------------
            for tt in list(range(4)) + list(range(4, KT)):
                ysb = y_pool.tile([128, C], F32, tag="y", name=f"y{tt}")
                for ec in range(EC):
                    py = proj_ps.tile([128, 384], F32, tag="mmps",
                                      name=f"py{tt}_{ec}")
                    for c in range(CT):
                        nc.tensor.matmul(
                            py,
                            outT[c][tt // 4][:, (tt % 4) * 128:(tt % 4 + 1) * 128],
                            wptt[c][ec],
                            start=(c == 0), stop=(c == CT - 1),
                        )
                    nc.vector.tensor_add(ysb[:, ec * 384:(ec + 1) * 384], py,
                                         bias_bc[:, ec * 384:(ec + 1) * 384])
                nc.sync.dma_start(out=y[tt * 128:(tt + 1) * 128, :], in_=ysb)

    nc.finalize()
    return nc


def _get_nc():
    global _NC_CACHE
    if _NC_CACHE is None:
        _NC_CACHE = _build()
    return _NC_CACHE


def _host_prep(x, Wqkv, Wproj, bproj, Aq, Bq, Av, Bv):
    """Fold LoRA + score scale into the weights; lay out and cast to bf16."""
    bf16 = ml_dtypes.bfloat16
    W = Wqkv.astype(np.float64)
    Wq = W[0:C].reshape(H, HD, C)
    Wk = W[C:2 * C].reshape(H, HD, C)
    Wv_ = W[2 * C:3 * C].reshape(H, HD, C)
    ABq = Aq.astype(np.float64) @ Bq.astype(np.float64)   # [HD, HD]
    ABv = Av.astype(np.float64) @ Bv.astype(np.float64)
    Wq = Wq + np.einsum('ed,hec->hdc', ABq, Wq)           # (I+AB).T @ Wq per head
    Wv_ = Wv_ + np.einsum('ed,hec->hdc', ABv, Wv_)
    Wq = Wq * (HD ** -0.5)                                # fold score scale

    # wqk[p, 0/1, c] = [K=c-rows(128), M = dims of head 2p (64) ++ head 2p+1 (64)]
    wqk = np.empty((P, 2, CT, 128, 128), np.float32)
    for p in range(P):
        for c in range(CT):
            cs = slice(c * 128, (c + 1) * 128)
            wqk[p, 0, c, :, 0:64] = Wq[2 * p][:, cs].T.astype(np.float32)
            wqk[p, 0, c, :, 64:128] = Wq[2 * p + 1][:, cs].T.astype(np.float32)
            wqk[p, 1, c, :, 0:64] = Wk[2 * p][:, cs].T.astype(np.float32)
            wqk[p, 1, c, :, 64:128] = Wk[2 * p + 1][:, cs].T.astype(np.float32)

    # wv[c] = [K=c-rows(128), all 768 v output features]
    WvT = Wv_.reshape(C, C).T.astype(np.float32)          # [c_in, v_out]
    wv = np.ascontiguousarray(WvT.reshape(CT, 128, C))

    # wpt[c] = Wproj.T c-tiles: [K=c(128), e(768)]
    WpT = Wproj.astype(np.float32).T                      # [c, e]
    wpt = np.ascontiguousarray(WpT.reshape(CT, 128, C))

    bias_ = bproj.astype(np.float32).reshape(1, C)

    wqk16 = wqk.astype(bf16)
    wv16 = wv.astype(bf16)
    wpt16 = wpt.astype(bf16)

    per_core = []
    for b in range(B):
        xTb = np.ascontiguousarray(x[b].astype(np.float32).T).astype(bf16)
        per_core.append({"xT": xTb, "wqk": wqk16, "wv": wv16, "wpt": wpt16,
                         "bias": bias_})
    return per_core


def kernel(x, Wqkv, Wproj, bproj, Aq, Bq, Av, Bv, _trace=False):
    x = np.asarray(x)
    in_maps = _host_prep(np.asarray(x), np.asarray(Wqkv), np.asarray(Wproj),
                         np.asarray(bproj), np.asarray(Aq), np.asarray(Bq),
                         np.asarray(Av), np.asarray(Bv))
    nc = _get_nc()
    res = run_bass_kernel_spmd(nc, in_maps, core_ids=list(range(N_CORES)),
                               trace=_trace)
    out = np.stack([res.results[b]["y"] for b in range(B)], axis=0)
    if _trace:
        kernel._last_result = res
    return out.astype(np.float32)


# revision 3
# speedup vs baseline: 1.1005x; 1.0016x over previous
"""LoRA attention kernel for Trainium2, batch-sharded across 8 NeuronCores.

Strategy (v3):
  - Data parallel: batch B=8 -> one batch element per core.
  - LoRA factors folded into Wqkv on the host (exact algebra, float64);
    the 1/sqrt(hd) score scale is folded into Wq as well.
  - All matmul operands are bfloat16 (1 cycle/row on the PE regardless of
    moving free dim, and FWL halves LDWEIGHTS time); accumulation is fp32
    in PSUM so only operand rounding is lost.
  - Heads are processed in pairs (2p, 2p+1). qT/kT tiles pack the pair's
    head dims on partitions [0:64] / [64:128]. The score matmuls are ROW
    TILED: two concurrent 64-row matmuls (tile_position auto-derived from
    base partitions) compute both heads' scores in one 512-column pass.
  - The pair's two score outputs go to one [128,1024] 2-bank PSUM tile so
    a single ACT instruction applies exp to both (halves ACT bubbles).
    The exp stream is the pacing engine (~107us); everything else is
    arranged to hide under it:
      * v-projection groups are interleaved with pair-0/qc-0 attention —
        the attention-value matmul for key tile kt only needs vaug[kt],
        so each vaug tile is produced just in time.
      * qk-projection for pair p+1 is emitted between qc0 and qc1 of
        pair p so its matmuls run in batches during ACT-paced gaps.
      * lead-in DMAs are split across the sync and gpsimd queues
        (descriptor posting on one queue serializes at ~0.6us each).
  - v is produced in natural layout with an extra all-ones column per head
    (65-col pitch); the attention-value matmul (M=65) accumulates softmax
    denominators for free in its last output row.
  - Normalization runs off the PE critical path: DVE drains av PSUM, the
    gpsimd DMA queue shifts the denominator row to partition 0, fast
    reciprocal, gpsimd partition-broadcast, DVE multiply into outT.
  - Output projection consumes outT (c-major, bf16) directly; bias fused
    into the PSUM drain; y is written back as bf16 (cast to f32 on host).
    Projection for the first query half is emitted right after the last
    pair so it fills PE gaps during the final ACT-paced stretch.
"""
import numpy as np
import ml_dtypes

import concourse.bass as bass
import concourse.bacc as bacc
import concourse.mybir as mybir
import concourse.tile as tile
from concourse.bass_utils import run_bass_kernel_spmd

F32 = mybir.dt.float32
BF16 = mybir.dt.bfloat16
EXP = mybir.ActivationFunctionType.Exp

B, N, C, H, HD = 8, 1024, 768, 12, 64
P = H // 2              # 6 head pairs
CT = C // 128           # 6 contraction tiles over C
QC = N // 512           # 2 query chunks of 512
KT = N // 128           # 8 key tiles of 128
EC = 2                  # output-projection feature chunks of 384
VP = HD + 1             # 65-col pitch per head in vaug
N_CORES = 8

_NC_CACHE = None


def _build():
    nc = bacc.Bacc(None, target_bir_lowering=False)

    xT = nc.dram_tensor("xT", [C, N], BF16, kind="ExternalInput")
    # wqk[p, 0/1(q/k), c] = [K=c-rows(128), M=(head 2p dims 64 | head 2p+1 dims 64)]
    wqk = nc.dram_tensor("wqk", [P, 2, CT, 128, 128], BF16, kind="ExternalInput")
    wv = nc.dram_tensor("wv", [CT, 128, C], BF16, kind="ExternalInput")
    wpt = nc.dram_tensor("wpt", [CT, 128, C], BF16, kind="ExternalInput")
    bias = nc.dram_tensor("bias", [1, C], F32, kind="ExternalInput")
    y = nc.dram_tensor("y", [N, C], BF16, kind="ExternalOutput")

    from contextlib import ExitStack
    with tile.TileContext(nc) as tc:
        with ExitStack() as ctx:
            pool = lambda name, bufs, **kw: ctx.enter_context(
                tc.tile_pool(name=name, bufs=bufs, **kw))
            xt_pool = pool("xt", 2 * CT)
            wqk_pool = pool("wqkp", 4 * CT)       # 2 pairs in flight x (q+k)
            w384_pool = pool("w384", 2 * CT)      # wv then wpt (disjoint phases)
            vaug_pool = pool("vaug", KT)
            qt_pool = pool("qtp", 4)              # qT pair tiles, 2 pairs
            kt_pool = pool("ktp", 4)
            et_pool = pool("expp", 6)
            avs_pool = pool("avsp", 4)
            iv_pool = pool("ivp", 4)
            bc_pool = pool("bcp", 3)
            ost_pool = pool("ostp", 3)
            out_pool = pool("outp", 2 * CT)
            y_pool = pool("yp", 3)
            cst_pool = pool("cst", 1)
            proj_ps = pool("proj_ps", 2, space="PSUM")
            sc_ps = pool("sc_ps", 2, space="PSUM")
            av_ps = pool("av_ps", 2, space="PSUM")

            # ---- PE warm-up: dummy matmuls bridge the DMA lead-in so the
            # HAM clock gate opens before real work arrives -----------------
            wuf = cst_pool.tile([128, 512], F32, tag="wuf")
            nc.vector.memset(wuf, 0.0)
            wur = cst_pool.tile([128, 512], BF16, tag="wur")
            nc.vector.tensor_copy(wur, wuf)

            def warmup(n, label):
                for i in range(n):
                    wps = proj_ps.tile([128, 512], F32, tag="mmps",
                                       name=f"wu{label}_{i}")
                    nc.tensor.matmul(wps, wur[:, 0:128], wur,
                                     start=True, stop=True)

            warmup(14, "a")

            # ---- loads ---------------------------------------------------
            def load_wqk(p, eng):
                wts = []
                for qk in range(2):
                    for c in range(CT):
                        wt = wqk_pool.tile([128, 128], BF16, tag="wqk",
                                           name=f"wqk{p}_{qk}_{c}")
                        eng.dma_start(out=wt, in_=wqk[p, qk, c, :, :])
                        wts.append(wt)
                return wts  # [q c0..c5, k c0..c5]

            wts0 = load_wqk(0, nc.sync)

            # x^T in 12 standalone [128, 512] bf16 tiles; qc0 on the sync
            # queue (critical for first scores), qc1 on gpsimd's queue
            xt = [[None] * QC for _ in range(CT)]
            for qc in range(QC):
                for c in range(CT):
                    t = xt_pool.tile([128, 512], BF16, tag="xt",
                                     name=f"xt{c}_{qc}")
                    eng = nc.sync if qc == 0 else nc.gpsimd
                    eng.dma_start(
                        out=t, in_=xT[c * 128:(c + 1) * 128,
                                      qc * 512:(qc + 1) * 512])
                    xt[c][qc] = t

            bias_bc = cst_pool.tile([128, C], F32, tag="biasbc")
            nc.gpsimd.dma_start(out=bias_bc,
                                in_=bias[:, :].to_broadcast([128, C]))
            ones12 = cst_pool.tile([128, H], BF16, tag="ones12")
            nc.vector.memset(ones12, 1.0)

            wvt = [[None] * 2 for _ in range(CT)]
            for c in range(CT):
                for half in range(2):
                    t = w384_pool.tile([128, 384], BF16, tag="w384",
                                       name=f"wv{c}_{half}")
                    nc.gpsimd.dma_start(
                        out=t, in_=wv[c, :, half * 384:(half + 1) * 384])
                    wvt[c][half] = t

            # ---- per-pair q/k projection (qc0 groups first so the first
            # scores can start two groups earlier) -------------------------
            def qk_project(p, wts):
                """qT_pair / kT_pair: [128 = (h dims | h' dims), 512 tokens]
                per 512-token chunk, bf16."""
                qts, kts = [None] * QC, [None] * QC
                for qc in range(QC):
                    for qk in range(2):
                        dst_pool = qt_pool if qk == 0 else kt_pool
                        st = dst_pool.tile([128, 512], BF16, tag="st",
                                           name=f"st{p}_{qk}_{qc}")
                        pqk = proj_ps.tile([128, 512], F32, tag="mmps",
                                           name=f"pqk{p}_{qk}_{qc}")
                        for c in range(CT):
                            nc.tensor.matmul(
                                pqk, wts[qk * CT + c], xt[c][qc],
                                start=(c == 0), stop=(c == CT - 1),
                            )
                        nc.vector.tensor_copy(st, pqk)
                        (qts if qk == 0 else kts)[qc] = st
                return qts, kts

            # ---- v_aug[kt] group emission (natural layout, [v | 1]) ------
            vaug = [None] * KT

            def vproj(tt):
                va = vaug_pool.tile([128, H * VP], BF16,
                                    tag="vaug", name=f"vaug{tt}")
                for half in range(2):
                    pv = proj_ps.tile([128, 384], F32, tag="mmps",
                                      name=f"pv{tt}_{half}")
                    for c in range(CT):
                        nc.tensor.matmul(
                            pv,
                            xt[c][tt // 4][:, (tt % 4) * 128:(tt % 4 + 1) * 128],
                            wvt[c][half],
                            start=(c == 0), stop=(c == CT - 1),
                        )
                    dst = bass.AP(tensor=va.tensor,
                                  offset=va.offset + half * 6 * VP,
                                  ap=[va.ap[0], [VP, 6], [1, HD]])
                    nc.vector.tensor_copy(dst, pv)
                ones_ap = bass.AP(tensor=va.tensor, offset=va.offset + HD,
                                  ap=[va.ap[0], [VP, H]])
                nc.vector.tensor_copy(ones_ap, ones12)
                vaug[tt] = va

            # ---- output accumulator tiles (c-major, [128, 512] per qc) ---
            outT = [[out_pool.tile([128, 512], BF16, tag="outT",
                                   name=f"outT{i}_{qc}")
                     for qc in range(QC)] for i in range(CT)]

            # ---- attention building blocks -------------------------------
            def attn_kt_step(p, qc, qts, kts, av0, av1, kt):
                h0, h1 = 2 * p, 2 * p + 1
                ps_s = sc_ps.tile([128, 1024], F32, tag="sc",
                                  name=f"sc{p}_{qc}_{kt}")
                klhs = kts[kt // 4][:, (kt % 4) * 128:(kt % 4 + 1) * 128]
                nc.tensor.matmul(
                    ps_s[:, 0:512], klhs[0:64, :], qts[qc][0:64, :],
                    start=True, stop=True,
                )
                nc.tensor.matmul(
                    ps_s[:, 512:1024], klhs[64:128, :], qts[qc][64:128, :],
                    start=True, stop=True,
                )
                et = et_pool.tile([128, 1024], BF16, tag="exp",
                                  name=f"exp{p}_{qc}_{kt}")
                nc.scalar.activation(out=et, in_=ps_s, func=EXP)
                nc.tensor.matmul(
                    av0, vaug[kt][:, h0 * VP:h0 * VP + VP], et[:, 0:512],
                    start=(kt == 0), stop=(kt == KT - 1),
                )
                nc.tensor.matmul(
                    av1, vaug[kt][:, h1 * VP:h1 * VP + VP], et[:, 512:1024],
                    start=(kt == 0), stop=(kt == KT - 1),
                )

            def attn_norm(p, qc, av0, av1):
                """Drain + normalize, off the PE critical path."""
                for hi, av in ((0, av0), (1, av1)):
                    h = 2 * p + hi
                    avs = avs_pool.tile([VP, 512], F32, tag="avs",
                                        name=f"avs{h}_{qc}")
                    nc.vector.tensor_copy(avs, av)
                    # row 64 = softmax denominators; shift to partition 0
                    sm0 = iv_pool.tile([1, 512], F32, tag="sm0",
                                       name=f"sm0{h}_{qc}")
                    nc.gpsimd.dma_start(out=sm0, in_=avs[HD:VP, :])
                    iv0 = iv_pool.tile([1, 512], F32, tag="iv0",
                                       name=f"iv0{h}_{qc}")
                    nc.vector.reciprocal_approx_fast(out=iv0, in_=sm0)
                    bc = bc_pool.tile([64, 512], F32, tag="bc",
                                      name=f"bc{h}_{qc}")
                    nc.gpsimd.partition_broadcast(bc, iv0)
                    if hi == 0:
                        nc.vector.tensor_mul(
                            outT[p][qc][0:64, :], avs[0:HD, :], bc)
                    else:
                        ost = ost_pool.tile([64, 512], BF16, tag="ost",
                                            name=f"ost{h}_{qc}")
                        nc.vector.tensor_mul(ost, avs[0:HD, :], bc)
                        nc.gpsimd.dma_start(out=outT[p][qc][64:128, :],
                                            in_=ost)

            def av_tiles(p, qc):
                h0, h1 = 2 * p, 2 * p + 1
                av0 = av_ps.tile([VP, 512], F32, tag="av", name=f"av{h0}_{qc}")
                av1 = av_ps.tile([VP, 512], F32, tag="av", name=f"av{h1}_{qc}")
                return av0, av1

            # ---- pipeline ------------------------------------------------
            # pair 0 q/k projection, then qc0 attention interleaved with the
            # v-projection (vaug[kt] is produced just before its av matmul).
            cur = qk_project(0, wts0)
            nxt_wts = load_wqk(1, nc.sync)

            qts, kts = cur
            av0, av1 = av_tiles(0, 0)
            for kt in range(KT):
                vproj(kt)
                attn_kt_step(0, 0, qts, kts, av0, av1, kt)
            attn_norm(0, 0, av0, av1)

            nxt = qk_project(1, nxt_wts)
            nxt_wts = load_wqk(2, nc.sync)

            av0, av1 = av_tiles(0, 1)
            for kt in range(KT):
                attn_kt_step(0, 1, qts, kts, av0, av1, kt)
            attn_norm(0, 1, av0, av1)

            wptt = None
            for p in range(1, P):
                qts, kts = nxt
                av0, av1 = av_tiles(p, 0)
                for kt in range(KT):
                    attn_kt_step(p, 0, qts, kts, av0, av1, kt)
                attn_norm(p, 0, av0, av1)

                # next pair's projection between the two query halves
                if p < P - 1:
                    nxt = qk_project(p + 1, nxt_wts)
                if p < P - 2:
                    nxt_wts = load_wqk(p + 2, nc.sync)
                if p == 2:
                    # prefetch output-projection weights mid-flight
                    wptt = [[None] * EC for _ in range(CT)]
                    for c in range(CT):
                        for ec in range(EC):
                            t = w384_pool.tile([128, 384], BF16, tag="w384",
                                               name=f"wpt{c}_{ec}")
                            nc.sync.dma_start(
                                out=t,
                                in_=wpt[c, :, ec * 384:(ec + 1) * 384])
                            wptt[c][ec] = t

                av0, av1 = av_tiles(p, 1)
                for kt in range(KT):
                    attn_kt_step(p, 1, qts, kts, av0, av1, kt)
                attn_norm(p, 1, av0, av1)

            # ---- output projection (qc0 tokens first: fills PE gaps while
            # the last pair's qc1 attention is still ACT-paced) ------------
            for tt in range(KT):
                ysb = y_pool.tile([128, C], BF16, tag="y", name=f"y{tt}")
                for ec in range(EC):
                    py = proj_ps.tile([128, 384], F32, tag="mmps",
                                      name=f"py{tt}_{ec}")
                    for c in range(CT):
                        nc.tensor.matmul(
                            py,
                            outT[c][tt // 4][:, (tt % 4) * 128:(tt % 4 + 1) * 128],
                            wptt[c][ec],
                            start=(c == 0), stop=(c == CT - 1),
                        )
                    nc.vector.tensor_add(ysb[:, ec * 384:(ec + 1) * 384], py,
                                         bias_bc[:, ec * 384:(ec + 1) * 384])
                nc.sync.dma_start(out=y[tt * 128:(tt + 1) * 128, :], in_=ysb)

    nc.finalize()
    return nc


def _get_nc():
    global _NC_CACHE
    if _NC_CACHE is None:
        _NC_CACHE = _build()
    return _NC_CACHE


def _host_prep(x, Wqkv, Wproj, bproj, Aq, Bq, Av, Bv):
    """Fold LoRA + score scale into the weights; lay out and cast to bf16."""
    bf16 = ml_dtypes.bfloat16
    W = Wqkv.astype(np.float64)
    Wq = W[0:C].reshape(H, HD, C)
    Wk = W[C:2 * C].reshape(H, HD, C)
    Wv_ = W[2 * C:3 * C].reshape(H, HD, C)
    ABq = Aq.astype(np.float64) @ Bq.astype(np.float64)   # [HD, HD]
    ABv = Av.astype(np.float64) @ Bv.astype(np.float64)
    Wq = Wq + np.einsum('ed,hec->hdc', ABq, Wq)           # (I+AB).T @ Wq per head
    Wv_ = Wv_ + np.einsum('ed,hec->hdc', ABv, Wv_)
    Wq = Wq * (HD ** -0.5)                                # fold score scale

    # wqk[p, 0/1, c] = [K=c-rows(128), M = dims of head 2p (64) ++ head 2p+1 (64)]
    wqk = np.empty((P, 2, CT, 128, 128), np.float32)
    for p in range(P):
        for c in range(CT):
            cs = slice(c * 128, (c + 1) * 128)
            wqk[p, 0, c, :, 0:64] = Wq[2 * p][:, cs].T.astype(np.float32)
            wqk[p, 0, c, :, 64:128] = Wq[2 * p + 1][:, cs].T.astype(np.float32)
            wqk[p, 1, c, :, 0:64] = Wk[2 * p][:, cs].T.astype(np.float32)
            wqk[p, 1, c, :, 64:128] = Wk[2 * p + 1][:, cs].T.astype(np.float32)

    # wv[c] = [K=c-rows(128), all 768 v output features]
    WvT = Wv_.reshape(C, C).T.astype(np.float32)          # [c_in, v_out]
    wv = np.ascontiguousarray(WvT.reshape(CT, 128, C))

    # wpt[c] = Wproj.T c-tiles: [K=c(128), e(768)]
    WpT = Wproj.astype(np.float32).T                      # [c, e]
    wpt = np.ascontiguousarray(WpT.reshape(CT, 128, C))

    bias_ = bproj.astype(np.float32).reshape(1, C)

    wqk16 = wqk.astype(bf16)
    wv16 = wv.astype(bf16)
    wpt16 = wpt.astype(bf16)

    per_core = []
    for b in range(B):
        xTb = np.ascontiguousarray(x[b].astype(np.float32).T).astype(bf16)
        per_core.append({"xT": xTb, "wqk": wqk16, "wv": wv16, "wpt": wpt16,
                         "bias": bias_})
    return per_core


def kernel(x, Wqkv, Wproj, bproj, Aq, Bq, Av, Bv, _trace=False):
    x = np.asarray(x)
    in_maps = _host_prep(np.asarray(x), np.asarray(Wqkv), np.asarray(Wproj),
                         np.asarray(bproj), np.asarray(Aq), np.asarray(Bq),
                         np.asarray(Av), np.asarray(Bv))
    nc = _get_nc()
    res = run_bass_kernel_spmd(nc, in_maps, core_ids=list(range(N_CORES)),
                               trace=_trace)
    out = np.stack([res.results[b]["y"] for b in range(B)], axis=0)
    if _trace:
        kernel._last_result = res
    return out.astype(np.float32)
